# revision 1
# baseline (speedup 1.0000x reference)
"""Fused linear + cross-entropy (mean NLL) on 8 trn2 NeuronCores.

Strategy: data-parallel over rows (1024 rows/core). Per core, compute
logits = e @ c.T + bias with rows on PSUM partitions and vocab on the
free dim, as 33 accumulating [K=128,M=128,N=512] matmuls per tile pair
(D padded 4096->4224: the extra K-chunk carries a constant-1 column in
e and the vocab bias as a column of c, folding the bias into the
matmul). ScalarE then does exp in one pass with accum_out producing the
per-row partial softmax sum for free. The target logit is a per-row dot
e[n]·c[t_n] done on DVE from a host-side gather. Host combines:
lse = log(sum of partials), nll = lse - (tgt_dot + bias[t]), mean.
"""

import json

import numpy as np
import ml_dtypes

import concourse.bass as bass
import concourse.tile as tile
from concourse import mybir
from concourse.bass_utils import run_bass_kernel_spmd
from concourse.tile import TileContext, ScopedClock

IGNORE_INDEX = -100

N, D, V = 8192, 4096, 50257
CORES = 8
R = N // CORES          # rows per core (1024)
RT = R // 128           # row tiles per core (8)
KC = 33                 # K chunks of 128 (D padded to 4224)
DP = KC * 128           # 4224
VTILE = 512
VT = 79 * 0 + (V + VTILE - 1) // VTILE  # 99
VP = VT * VTILE         # 50688
BPAD = -40.0            # bias for padded vocab rows -> exp ~ 0

BF16 = ml_dtypes.bfloat16

_PATCHED = False


def _patch_tile_drain():
    """This container's walrus rejects >1 sync-wait on a CTRL instruction;
    Tile's tail drain carries one wait per live semaphore. Split them into
    single-wait wait_ge ops on the sync queue."""
    global _PATCHED
    if _PATCHED:
        return
    _PATCHED = True

    # This walrus build accepts at most ONE sync-wait per instruction.
    # Post-process the serialized BIR: hoist extra waits onto same-engine
    # NoOps inserted directly before the instruction (identical sync
    # semantics - the engine stalls on each nop in turn).
    orig_to_json = bass.Bass.to_json_bytes

    def to_json_bytes_split(self, *a, **kw):
        m = json.loads(orig_to_json(self, *a, **kw))
        for f in m.get("functions", []):
            for blk in f.get("blocks", []):
                out = []
                for ins in blk["instructions"]:
                    w = (ins.get("sync_info") or {}).get("on_wait") or []
                    if len(w) > 1:
                        for i, wi in enumerate(w[:-1]):
                            out.append(
                                {
                                    "debug": ins.get("debug"),
                                    "engine": ins["engine"],
                                    "ins": [],
                                    "name": f"{ins['name']}-sw{i}",
                                    "opcode": "NoOp",
                                    "outs": [],
                                    "sync_info": {
                                        "on_update": [],
                                        "on_wait": [wi],
                                    },
                                }
                            )
                        ins["sync_info"]["on_wait"] = [w[-1]]
                    out.append(ins)
                blk["instructions"] = out
        return json.dumps(m).encode()

    bass.Bass.to_json_bytes = to_json_bytes_split

    def _drain_and_barrier(self, tick_clock, wait_clock):
        nc = self.nc
        probe = nc.sync.nop(nofuse=True)
        wait_clock.add_sem_waits(
            probe.ins, ScopedClock({None: tick_clock.global_clock})
        )
        waits = list(probe.ins.sync_info.on_wait)
        probe.ins.sync_info.on_wait = []
        by_name = {h.name: h for h in self.sems.allocated().values()}
        for w in waits:
            nc.sync.wait_ge(by_name[w.ant_name], w.wait_value)
        nc.sync.drain()
        nc.all_engine_barrier()
        popped = nc._tile_sem_poison_stack.pop()
        assert popped is self._sem_poison
        nc.clear_and_free_semaphores(list(self.sems.allocated().values()))
        nc.all_engine_barrier()

    TileContext._drain_and_barrier = _drain_and_barrier


# This walrus/runtime rejects NEFFs beyond ~16k PE instructions
# (26k-matmul single NEFF dies NRT_EXEC_UNIT_UNRECOVERABLE; 13.2k is
# proven good), so the 99-v-tile sweep runs as 2 sequential launches.
VT_GROUPS = [50, 49]

_NC_CACHE = {}


def _build_module(vt_n):
    if vt_n in _NC_CACHE:
        return _NC_CACHE[vt_n]
    _patch_tile_drain()
    f32 = mybir.dt.float32
    bf = mybir.dt.bfloat16
    nc = bass.Bass("TRN2")
    ct_d = nc.dram_tensor("ct", [vt_n, 128, KC, VTILE], bf, kind="ExternalInput")
    et_d = nc.dram_tensor("et", [128, RT, KC, 128], bf, kind="ExternalInput")
    sume_d = nc.dram_tensor("sume", [RT, 128, vt_n], f32, kind="ExternalOutput")

    with TileContext(nc) as tc:
        with (
            tc.tile_pool(name="singles", bufs=1) as singles,
            tc.tile_pool(name="ctp", bufs=2) as ctp,
            tc.tile_pool(name="ex", bufs=4) as exp_pool,
            tc.tile_pool(name="psum", bufs=6, space="PSUM") as psum,
        ):
            et_sb = singles.tile([128, RT, KC, 128], bf)
            nc.sync.dma_start(out=et_sb, in_=et_d[:, :, :, :])
            acc3 = singles.tile([128, RT, vt_n], f32)

            for vt in range(vt_n):
                ct_t = ctp.tile([128, KC, VTILE], bf, tag="ct")
                nc.sync.dma_start(out=ct_t, in_=ct_d[vt])
                for r in range(RT):
                    ps = psum.tile([128, VTILE], mybir.dt.float32, tag="ps")
                    for k in range(KC):
                        nc.tensor.matmul(
                            ps,
                            et_sb[:, r, k, :],
                            ct_t[:, k, :],
                            start=(k == 0),
                            stop=(k == KC - 1),
                        )
                    ex_t = exp_pool.tile([128, VTILE], bf, tag="ex")
                    nc.scalar.activation(
                        out=ex_t,
                        in_=ps,
                        func=mybir.ActivationFunctionType.Exp,
                        scale=1.0,
                        accum_out=acc3[:, r, vt : vt + 1],
                    )

            for r in range(RT):
                nc.gpsimd.dma_start(out=sume_d[r], in_=acc3[:, r, :])

    _NC_CACHE[vt_n] = nc
    return nc


def _prep_inputs(e, c, bias, targets):
    e_np = np.asarray(e, dtype=np.float32)
    c_np = np.asarray(c, dtype=np.float32)
    bias_np = np.asarray(bias, dtype=np.float32)
    t_np = np.asarray(targets).astype(np.int64)
    valid = t_np != IGNORE_INDEX
    safe_t = np.where(valid, t_np, 0)

    # e augmented with a constant-1 column in the pad K-chunk
    e_aug = np.zeros((N, DP), dtype=BF16)
    e_aug[:, :D] = e_np.astype(BF16)
    e_aug[:, D] = 1.0

    # c augmented: bias as column D; pad vocab rows get bias BPAD
    c_aug = np.zeros((VP, DP), dtype=BF16)
    c_aug[:V, :D] = c_np.astype(BF16)
    c_aug[:V, D] = bias_np.astype(BF16)
    c_aug[V:, D] = BPAD
    # -> [VT, 128(kp), KC, VTILE] : ct[vt, kp, k, vc] = c_aug[vt*512+vc, k*128+kp]
    ct = np.ascontiguousarray(
        c_aug.reshape(VT, VTILE, KC, 128).transpose(0, 3, 2, 1)
    )
    del c_aug

    # target logit: the one gathered dot-product per row, done host-side in
    # full f32 (0.001% of the problem's FLOPs)
    tgt_host = np.einsum("nd,nd->n", e_np, c_np[safe_t]).astype(np.float64)

    in_maps = []
    for cid in range(CORES):
        sl = slice(cid * R, (cid + 1) * R)
        et = np.ascontiguousarray(
            e_aug[sl].reshape(RT, 128, KC, 128).transpose(3, 0, 2, 1)
        )  # [128, RT, KC, 128]
        in_maps.append({"ct": ct, "et": et})
    return in_maps, valid, safe_t, bias_np, tgt_host


def _combine(results, valid, safe_t, bias_np, tgt_host):
    S = np.stack([r["sume"] for r in results])  # [CORES, RT, 128, VT]
    S = S.astype(np.float64).sum(axis=-1).reshape(N)
    lse = np.log(S)
    tgt_logit = tgt_host + bias_np.astype(np.float64)[safe_t]
    nll = np.where(valid, lse - tgt_logit, 0.0)
    n_valid = max(int(valid.sum()), 1)
    return np.float32(nll.sum() / n_valid)


def kernel(e, c, bias, targets, _trace=False):
    in_maps, valid, safe_t, bias_np, tgt_host = _prep_inputs(e, c, bias, targets)
    merged = [
        {"sume": np.empty((RT, 128, VT), np.float32)} for _ in range(CORES)
    ]
    g0 = 0
    kernel.last_run_wall_s = 0.0
    for vt_n in VT_GROUPS:
        nc = _build_module(vt_n)
        group_maps = [
            {"ct": m["ct"][g0 : g0 + vt_n], "et": m["et"]} for m in in_maps
        ]
        import time as _time

        _t = _time.time()
        res = run_bass_kernel_spmd(
            nc, group_maps, core_ids=list(range(CORES)), trace=False
        )
        kernel.last_run_wall_s += _time.time() - _t
        for cid in range(CORES):
            merged[cid]["sume"][:, :, g0 : g0 + vt_n] = res.results[cid]["sume"]
        g0 += vt_n
    return _combine(merged, valid, safe_t, bias_np, tgt_host)



# revision 2
# speedup vs baseline: 16.9458x; 16.9458x over previous
"""Fused linear + cross-entropy (mean NLL) on 8 trn2 NeuronCores.

Strategy: vocab-parallel. Core c owns vocab rows [c*6656, (c+1)*6656) of
a 53248-padded vocab and computes, for ALL 8192 rows, the partial
softmax sums over its vocab shard. Both operands are fp8e4 (scaled by
16; logits come out x256, undone by the Exp activation's scale=1/256)
and matmuls run in DoubleRow perf mode (K=256 per instruction, 2x PE
throughput). e is shipped once as 8 shards and AllGathered on device;
c's shard + a tiny fp8 bias vector are shipped per core, so total H2D
is ~255MB instead of the 3.4GB a data-parallel bf16 layout needs. The
vocab bias is folded in as a 17th contraction chunk whose operands are
built on device (memset + one 6.6KB DMA), not shipped. Padded vocab
rows are all-zero -> each contributes exp(0)=1 to the partial sum,
subtracted exactly on host. The target logit e[n]·c[t_n] is a host-side
f32 einsum (0.001% of FLOPs). Host combines: lse = log(sum of partials
- n_pad), nll = lse - (tgt + bias[t]), mean over valid rows.
"""

import json

import numpy as np
import ml_dtypes

import concourse.bass as bass
import concourse.tile as tile
from concourse import mybir
from concourse.bass_utils import run_bass_kernel_spmd
from concourse.tile import TileContext, ScopedClock

IGNORE_INDEX = -100

N, D, V = 8192, 4096, 50257
CORES = 8
VTILE = 512
VTC = 13                 # vocab tiles per core
VPC = VTC * VTILE        # vocab per core (6656)
VP = VPC * CORES         # padded vocab (53248)
NPAD = VP - V            # 2991 pad rows, all in core 7
KC2 = 16                 # double-K chunks (D = 16*256 exactly)
RT = N // 128            # 64 row tiles
RG = 8                   # row groups
RTG = RT // RG           # row tiles per group (8)
SCALE = 16.0             # fp8 input scale; logits scale by 256
ESH = 128 // CORES       # K-partition rows per e-shard (16)

F8 = ml_dtypes.float8_e4m3

_PATCHED = False


def _patch_tile_drain():
    """This container's walrus rejects >1 sync-wait on a CTRL instruction;
    Tile's tail drain carries one wait per live semaphore. Split them into
    single-wait wait_ge ops on the sync queue."""
    global _PATCHED
    if _PATCHED:
        return
    _PATCHED = True

    # This walrus build accepts at most ONE sync-wait per instruction.
    # Post-process the serialized BIR: hoist extra waits onto same-engine
    # NoOps inserted directly before the instruction (identical sync
    # semantics - the engine stalls on each nop in turn).
    orig_to_json = bass.Bass.to_json_bytes

    def to_json_bytes_split(self, *a, **kw):
        m = json.loads(orig_to_json(self, *a, **kw))
        for f in m.get("functions", []):
            for blk in f.get("blocks", []):
                out = []
                for ins in blk["instructions"]:
                    w = (ins.get("sync_info") or {}).get("on_wait") or []
                    if len(w) > 1:
                        for i, wi in enumerate(w[:-1]):
                            out.append(
                                {
                                    "debug": ins.get("debug"),
                                    "engine": ins["engine"],
                                    "ins": [],
                                    "name": f"{ins['name']}-sw{i}",
                                    "opcode": "NoOp",
                                    "outs": [],
                                    "sync_info": {
                                        "on_update": [],
                                        "on_wait": [wi],
                                    },
                                }
                            )
                        ins["sync_info"]["on_wait"] = [w[-1]]
                    out.append(ins)
                blk["instructions"] = out
        return json.dumps(m).encode()

    bass.Bass.to_json_bytes = to_json_bytes_split

    def _drain_and_barrier(self, tick_clock, wait_clock):
        nc = self.nc
        probe = nc.sync.nop(nofuse=True)
        wait_clock.add_sem_waits(
            probe.ins, ScopedClock({None: tick_clock.global_clock})
        )
        waits = list(probe.ins.sync_info.on_wait)
        probe.ins.sync_info.on_wait = []
        by_name = {h.name: h for h in self.sems.allocated().values()}
        for w in waits:
            nc.sync.wait_ge(by_name[w.ant_name], w.wait_value)
        nc.sync.drain()
        nc.all_engine_barrier()
        popped = nc._tile_sem_poison_stack.pop()
        assert popped is self._sem_poison
        nc.clear_and_free_semaphores(list(self.sems.allocated().values()))
        nc.all_engine_barrier()

    TileContext._drain_and_barrier = _drain_and_barrier


_NC_CACHE = {}


def _build_module():
    if "nc" in _NC_CACHE:
        return _NC_CACHE["nc"]
    _patch_tile_drain()
    f32 = mybir.dt.float32
    f8 = mybir.dt.float8e4
    bf = mybir.dt.bfloat16
    DR = mybir.MatmulPerfMode.DoubleRow

    nc = bass.Bass("TRN2", num_devices=CORES)
    # ct[vt, kp, kc2, two, vc] = c8[core_vocab + vt*512 + vc, kc2*256 + two*128 + kp]
    ct_d = nc.dram_tensor("ct", [VTC, 128, KC2, 2, VTILE], f8, kind="ExternalInput")
    # esh[kp_local, rt, kc2, two, m]: this core's 16-partition slab of the
    # full et[kp, rt, kc2, two, m] = e8[rt*128 + m, kc2*256 + two*128 + kp]
    esh_d = nc.dram_tensor("esh", [ESH, RT, KC2, 2, 128], f8, kind="ExternalInput")
    bias_d = nc.dram_tensor("biasv", [VTC, VTILE], f8, kind="ExternalInput")
    sume_d = nc.dram_tensor("sume", [128, RT, VTC], f32, kind="ExternalOutput")

    with TileContext(nc) as tc:
        with (
            tc.tile_pool(name="dram", bufs=1, space="DRAM") as dram,
            tc.tile_pool(name="singles", bufs=1) as singles,
            tc.tile_pool(name="etp", bufs=2) as etp,
            tc.tile_pool(name="ctp", bufs=2) as ctp,
            tc.tile_pool(name="ex", bufs=2) as exp_pool,
            tc.tile_pool(name="psum", bufs=6, space="PSUM") as psum,
        ):
            # --- AllGather e: each core ships 1/8 of et, gathers the rest ---
            e_bounce = dram.tile([ESH, RT, KC2, 2, 128], f8)
            et_full = dram.tile([128, RT, KC2, 2, 128], f8, addr_space="Shared")
            nc.gpsimd.dma_start(e_bounce[:], esh_d[:, :, :, :, :])
            nc.gpsimd.collective_compute(
                "AllGather",
                mybir.AluOpType.bypass,
                replica_groups=[list(range(CORES))],
                ins=[e_bounce.opt()],
                outs=[et_full.opt()],
            )

            # --- bias chunk operands, built on device ---
            # lhsT for the bias matmul: [128, 2, 128], only [0, 0, :] = SCALE
            e_bias = singles.tile([128, 2, 128], f8)
            nc.vector.memset(e_bias[:, :, :], 0)
            nc.vector.memset(e_bias[0:1, 0, :], SCALE)
            # rhs: [128, vt, 2, 512], only partition 0, two=0 holds 16*bias
            bias_sb = singles.tile([128, VTC, 2, VTILE], f8)
            nc.vector.memset(bias_sb[:, :, :, :], 0)
            nc.sync.dma_start(out=bias_sb[0:1, :, 0, :], in_=bias_d[:, :])

            acc = singles.tile([128, RT, VTC], f32)

            for rg in range(RG):
                et_t = etp.tile([128, RTG, KC2, 2, 128], f8, tag="et")
                nc.sync.dma_start(
                    out=et_t, in_=et_full[:, rg * RTG : (rg + 1) * RTG]
                )
                for vt in range(VTC):
                    ct_t = ctp.tile([128, KC2, 2, VTILE], f8, tag="ct")
                    nc.sync.dma_start(out=ct_t, in_=ct_d[vt])
                    for rt in range(RTG):
                        ps = psum.tile([128, VTILE], f32, tag="ps")
                        for k in range(KC2):
                            nc.tensor.matmul(
                                ps,
                                et_t[:, rt, k, :, :],
                                ct_t[:, k, :, :],
                                start=(k == 0),
                                stop=False,
                                perf_mode=DR,
                            )
                        nc.tensor.matmul(
                            ps,
                            e_bias[:, :, :],
                            bias_sb[:, vt, :, :],
                            start=False,
                            stop=True,
                            perf_mode=DR,
                        )
                        ex_t = exp_pool.tile([128, VTILE], bf, tag="ex")
                        nc.scalar.activation(
                            out=ex_t,
                            in_=ps,
                            func=mybir.ActivationFunctionType.Exp,
                            scale=1.0 / (SCALE * SCALE),
                            accum_out=acc[:, rg * RTG + rt, vt : vt + 1],
                        )

            nc.gpsimd.dma_start(out=sume_d[:, :, :], in_=acc[:, :, :])

    _NC_CACHE["nc"] = nc
    return nc


def _prep_inputs(e, c, bias, targets):
    e_np = np.asarray(e, dtype=np.float32)
    c_np = np.asarray(c, dtype=np.float32)
    bias_np = np.asarray(bias, dtype=np.float32)
    t_np = np.asarray(targets).astype(np.int64)
    valid = t_np != IGNORE_INDEX
    safe_t = np.where(valid, t_np, 0)

    # e: scale, cast, tile to [kp, rt, kc2, two, m], shard along kp
    e8 = (e_np * SCALE).astype(F8)
    et = np.ascontiguousarray(
        e8.reshape(RT, 128, KC2, 2, 128).transpose(4, 0, 2, 3, 1)
    )  # [128, RT, KC2, 2, 128]
    esh = et.reshape(CORES, ESH, RT, KC2, 2, 128)

    # c: scale, cast, pad vocab with zeros, tile per core
    c8 = np.zeros((VP, D), dtype=F8)
    c8[:V] = (c_np * SCALE).astype(F8)
    ct = np.ascontiguousarray(
        c8.reshape(CORES, VTC, VTILE, KC2, 2, 128).transpose(0, 1, 5, 3, 4, 2)
    )  # [CORES, VTC, 128, KC2, 2, VTILE]

    b8 = np.zeros((VP,), dtype=F8)
    b8[:V] = (bias_np * SCALE).astype(F8)
    bt = b8.reshape(CORES, VTC, VTILE)

    # target logit: one gathered dot-product per row, host-side f32
    tgt_host = np.einsum("nd,nd->n", e_np, c_np[safe_t]).astype(np.float64)

    in_maps = [
        {"ct": ct[cid], "esh": esh[cid], "biasv": bt[cid]} for cid in range(CORES)
    ]
    return in_maps, valid, safe_t, bias_np, tgt_host


def _combine(results, valid, safe_t, bias_np, tgt_host):
    S = np.stack([r["sume"] for r in results])  # [CORES, 128, RT, VTC]
    S = S.astype(np.float64).sum(axis=(0, 3))  # [128, RT]
    S = S.T.reshape(N) - NPAD  # row n = rt*128 + p; drop exp(0)=1 pads
    lse = np.log(S)
    tgt_logit = tgt_host + bias_np.astype(np.float64)[safe_t]
    nll = np.where(valid, lse - tgt_logit, 0.0)
    n_valid = max(int(valid.sum()), 1)
    return np.float32(nll.sum() / n_valid)


def kernel(e, c, bias, targets, _trace=False):
    import time as _time

    _t0 = _time.time()
    in_maps, valid, safe_t, bias_np, tgt_host = _prep_inputs(e, c, bias, targets)
    kernel.last_prep_wall_s = _time.time() - _t0
    nc = _build_module()
    kernel.last_build_wall_s = _time.time() - _t0 - kernel.last_prep_wall_s
    _t = _time.time()
    res = run_bass_kernel_spmd(
        nc, in_maps, core_ids=list(range(CORES)), trace=False
    )
    kernel.last_run_wall_s = _time.time() - _t
    return _combine(res.results, valid, safe_t, bias_np, tgt_host)


# revision 3
# speedup vs baseline: 27.9513x; 1.6495x over previous
"""Fused linear + cross-entropy (mean NLL) on 8 trn2 NeuronCores.

Strategy: vocab-parallel. Core c owns vocab rows [c*6656, (c+1)*6656) of
a 53248-padded vocab and computes, for ALL 8192 rows, the partial
softmax sums over its vocab shard.

Numerics: e ships as fp8e4 scaled x16; c ships as int4 codes (two per
byte, mid-rise quantizer with step = 0.34*std(c)), unpacked on device
by DVE (AND/SHR) + DVE/ACT casts into fp8 code values 0..15. Matmuls
run fp8 DoubleRow (K=256/instruction, 2x PE rate); the accumulator
holds 16*sum(e_hat*code), which the Exp activation rescales by
step/16. The int4 zero-offset (-7.5*step per c element) contributes a
per-row constant absorbed into lse on host via the quantized e rowsum.
The vocab bias is folded in as a 17th contraction chunk (values
bias/step, fp8) whose operands are built on device. Padded vocab rows
are code 0 with zero bias -> each contributes exp(0)=1, subtracted
exactly on host. The target logit e[n]·c[t_n] is a host-side f32
einsum. Host combines: lse = log(partials - n_pad) - 7.5*step*rowsum,
nll = lse - (tgt + bias[t]), mean over valid rows.

Transfers dominate wall time (the axon tunnel moves ~28 MB/s), so the
kernel ships ~150MB total (int4 c shards 109MB + fp8 e 34MB) with
device_puts issued asynchronously per core, overlapping the upload
with host prep, module build, and the walrus compile; e is shipped as
8 disjoint shards and AllGathered on device.
"""

import json
import time

import numpy as np
import ml_dtypes

import jax
import jax.numpy as jnp
from jax.sharding import Mesh, NamedSharding, PartitionSpec

import concourse.bass as bass
import concourse.tile as tile
from concourse import mybir
from concourse.bass_utils import run_bass_kernel_spmd
from concourse.tile import TileContext, ScopedClock

IGNORE_INDEX = -100

N, D, V = 8192, 4096, 50257
CORES = 8
VTILE = 512
VTC = 13                 # vocab tiles per core
VPC = VTC * VTILE        # vocab per core (6656)
VP = VPC * CORES         # padded vocab (53248)
NPAD = VP - V            # 2991 pad rows, all in core 7
KC2 = 16                 # double-K chunks (D = 16*256 exactly)
KPK = KC2 // 2           # packed chunk-pairs (8)
RT = N // 128            # 64 row tiles
RG = 8                   # row groups
RTG = RT // RG           # row tiles per group (8)
SCALE = 16.0             # fp8 e scale; matmul acc = 16*sum(e_hat*code)
ESH = 128 // CORES       # K-partition rows per e-shard (16)

F8 = ml_dtypes.float8_e4m3

_PATCHED = False


def _patch_tile_drain():
    """This container's walrus rejects >1 sync-wait on a CTRL instruction;
    Tile's tail drain carries one wait per live semaphore. Split them into
    single-wait wait_ge ops on the sync queue."""
    global _PATCHED
    if _PATCHED:
        return
    _PATCHED = True

    # This walrus build accepts at most ONE sync-wait per instruction.
    # Post-process the serialized BIR: hoist extra waits onto same-engine
    # NoOps inserted directly before the instruction (identical sync
    # semantics - the engine stalls on each nop in turn).
    orig_to_json = bass.Bass.to_json_bytes

    def to_json_bytes_split(self, *a, **kw):
        m = json.loads(orig_to_json(self, *a, **kw))
        for f in m.get("functions", []):
            for blk in f.get("blocks", []):
                out = []
                for ins in blk["instructions"]:
                    w = (ins.get("sync_info") or {}).get("on_wait") or []
                    if len(w) > 1:
                        for i, wi in enumerate(w[:-1]):
                            out.append(
                                {
                                    "debug": ins.get("debug"),
                                    "engine": ins["engine"],
                                    "ins": [],
                                    "name": f"{ins['name']}-sw{i}",
                                    "opcode": "NoOp",
                                    "outs": [],
                                    "sync_info": {
                                        "on_update": [],
                                        "on_wait": [wi],
                                    },
                                }
                            )
                        ins["sync_info"]["on_wait"] = [w[-1]]
                    out.append(ins)
                blk["instructions"] = out
        return json.dumps(m).encode()

    bass.Bass.to_json_bytes = to_json_bytes_split

    def _drain_and_barrier(self, tick_clock, wait_clock):
        nc = self.nc
        probe = nc.sync.nop(nofuse=True)
        wait_clock.add_sem_waits(
            probe.ins, ScopedClock({None: tick_clock.global_clock})
        )
        waits = list(probe.ins.sync_info.on_wait)
        probe.ins.sync_info.on_wait = []
        by_name = {h.name: h for h in self.sems.allocated().values()}
        for w in waits:
            nc.sync.wait_ge(by_name[w.ant_name], w.wait_value)
        nc.sync.drain()
        nc.all_engine_barrier()
        popped = nc._tile_sem_poison_stack.pop()
        assert popped is self._sem_poison
        nc.clear_and_free_semaphores(list(self.sems.allocated().values()))
        nc.all_engine_barrier()

    TileContext._drain_and_barrier = _drain_and_barrier


_NC_CACHE = {}


def _build_module(step):
    if step in _NC_CACHE:
        return _NC_CACHE[step]
    _patch_tile_drain()
    f32 = mybir.dt.float32
    f8 = mybir.dt.float8e4
    u8 = mybir.dt.uint8
    bf = mybir.dt.bfloat16
    DR = mybir.MatmulPerfMode.DoubleRow
    AND = mybir.AluOpType.bitwise_and
    SHR = mybir.AluOpType.logical_shift_right
    MUL = mybir.AluOpType.mult

    nc = bass.Bass("TRN2", num_devices=CORES)
    # packed ct[vt, kp, kpk, two, vc]: low nibble = code of kc2=kpk, high
    # nibble = code of kc2=kpk+8, where
    # code[v, d] at d = kc2*256 + two*128 + kp, v = core_vocab + vt*512 + vc
    ct_d = nc.dram_tensor("ct", [VTC, 128, KPK, 2, VTILE], u8, kind="ExternalInput")
    # esh[kp_local, rt, kc2, two, m]: this core's 16-partition slab of the
    # full et[kp, rt, kc2, two, m] = e8[rt*128 + m, kc2*256 + two*128 + kp]
    esh_d = nc.dram_tensor("esh", [ESH, RT, KC2, 2, 128], f8, kind="ExternalInput")
    bias_d = nc.dram_tensor("biasv", [VTC, VTILE], f8, kind="ExternalInput")
    sume_d = nc.dram_tensor("sume", [128, RT, VTC], f32, kind="ExternalOutput")

    with TileContext(nc) as tc:
        with (
            tc.tile_pool(name="dram", bufs=1, space="DRAM") as dram,
            tc.tile_pool(name="singles", bufs=1) as singles,
            tc.tile_pool(name="etp", bufs=2) as etp,
            tc.tile_pool(name="ctpk", bufs=2) as ctpk,
            tc.tile_pool(name="ctu", bufs=2) as ctu,
            tc.tile_pool(name="ctp", bufs=2) as ctp,
            tc.tile_pool(name="ex", bufs=2) as exp_pool,
            tc.tile_pool(name="psum", bufs=6, space="PSUM") as psum,
        ):
            # --- AllGather e: each core ships 1/8 of et, gathers the rest ---
            e_bounce = dram.tile([ESH, RT, KC2, 2, 128], f8)
            et_full = dram.tile([128, RT, KC2, 2, 128], f8, addr_space="Shared")
            nc.gpsimd.dma_start(e_bounce[:], esh_d[:, :, :, :, :])
            nc.gpsimd.collective_compute(
                "AllGather",
                mybir.AluOpType.bypass,
                replica_groups=[list(range(CORES))],
                ins=[e_bounce.opt()],
                outs=[et_full.opt()],
            )

            # --- bias chunk operands, built on device ---
            # lhsT for the bias matmul: [128, 2, 128], only [0, 0, :] = SCALE
            e_bias = singles.tile([128, 2, 128], f8)
            nc.vector.memset(e_bias[:, :, :], 0)
            nc.vector.memset(e_bias[0:1, 0, :], SCALE)
            # rhs: [128, vt, 2, 512], only partition 0, two=0 holds bias/step
            bias_sb = singles.tile([128, VTC, 2, VTILE], f8)
            nc.vector.memset(bias_sb[:, :, :, :], 0)
            nc.sync.dma_start(out=bias_sb[0:1, :, 0, :], in_=bias_d[:, :])

            acc = singles.tile([128, RT, VTC], f32)

            for rg in range(RG):
                et_t = etp.tile([128, RTG, KC2, 2, 128], f8, tag="et")
                nc.sync.dma_start(
                    out=et_t, in_=et_full[:, rg * RTG : (rg + 1) * RTG]
                )
                for vt in range(VTC):
                    pk_t = ctpk.tile([128, KPK, 2, VTILE], u8, tag="pk")
                    nc.sync.dma_start(out=pk_t, in_=ct_d[vt])
                    lo_u = ctu.tile([128, KPK, 2, VTILE], u8, tag="lo")
                    hi_u = ctu.tile([128, KPK, 2, VTILE], u8, tag="hi")
                    nc.vector.tensor_scalar(
                        out=lo_u, in0=pk_t, scalar1=0x0F, scalar2=None, op0=AND
                    )
                    nc.vector.tensor_scalar(
                        out=hi_u, in0=pk_t, scalar1=4, scalar2=None, op0=SHR
                    )
                    ct_t = ctp.tile([128, KC2, 2, VTILE], f8, tag="ct")
                    nc.vector.tensor_scalar(
                        out=ct_t[:, 0:KPK], in0=lo_u, scalar1=1.0,
                        scalar2=None, op0=MUL,
                    )
                    nc.scalar.copy(out=ct_t[:, KPK:KC2], in_=hi_u)
                    for rt in range(RTG):
                        ps = psum.tile([128, VTILE], f32, tag="ps")
                        for k in range(KC2):
                            nc.tensor.matmul(
                                ps,
                                et_t[:, rt, k, :, :],
                                ct_t[:, k, :, :],
                                start=(k == 0),
                                stop=False,
                                perf_mode=DR,
                            )
                        nc.tensor.matmul(
                            ps,
                            e_bias[:, :, :],
                            bias_sb[:, vt, :, :],
                            start=False,
                            stop=True,
                            perf_mode=DR,
                        )
                        ex_t = exp_pool.tile([128, VTILE], bf, tag="ex")
                        nc.scalar.activation(
                            out=ex_t,
                            in_=ps,
                            func=mybir.ActivationFunctionType.Exp,
                            scale=float(step) / SCALE,
                            accum_out=acc[:, rg * RTG + rt, vt : vt + 1],
                        )

            nc.gpsimd.dma_start(out=sume_d[:, :, :], in_=acc[:, :, :])

    _NC_CACHE[step] = nc
    return nc


def _pack_c(c, inv_step):
    codes = jnp.clip(jnp.floor(c * inv_step) + 8.0, 0.0, 15.0).astype(jnp.uint8)
    codes = jnp.pad(codes, ((0, VP - V), (0, 0)))  # pads get code 0
    t = codes.reshape(CORES, VTC, VTILE, KC2, 2, 128).transpose(0, 1, 5, 3, 4, 2)
    return t[:, :, :, 0:KPK] | (t[:, :, :, KPK:KC2] << 4)


def _tgt_dot(e, crows):
    return jnp.sum(e.astype(jnp.float32) * crows.astype(jnp.float32), axis=1)


def _sharded_exec(nc, mesh, named_global, zeros_global, timers):
    """Adapted from bass2jax.run_bass_via_pjrt: execute a prebuilt Bass
    module on pre-sharded device-resident jax Arrays (no host concat,
    no implicit H2D at dispatch)."""
    from concourse.bass2jax import _bass_exec_p, partition_id_tensor

    partition_name = (
        nc.partition_id_tensor.name if nc.partition_id_tensor else None
    )
    in_names = []
    out_names = []
    out_avals = []
    for alloc in nc.m.functions[0].allocations:
        if not isinstance(alloc, mybir.MemoryLocationSet):
            continue
        name = alloc.memorylocations[0].name
        if alloc.kind == "ExternalInput":
            if name != partition_name:
                in_names.append(name)
        elif alloc.kind == "ExternalOutput":
            out_names.append(name)
            out_avals.append(
                jax.core.ShapedArray(
                    tuple(alloc.tensor_shape), mybir.dt.np(alloc.dtype)
                )
            )
    n_params = len(in_names)
    n_outs = len(out_avals)
    in_names = in_names + out_names
    if partition_name is not None:
        in_names.append(partition_name)

    def _body(*args):
        operands = list(args)
        if partition_name is not None:
            operands.append(partition_id_tensor())
        outs = _bass_exec_p.bind(
            *operands,
            out_avals=tuple(out_avals),
            in_names=tuple(in_names),
            out_names=tuple(out_names),
            lowering_input_output_aliases=(),
            sim_require_finite=True,
            sim_require_nnan=True,
            nc=nc,
        )
        return tuple(outs)

    donate = tuple(range(n_params, n_params + n_outs))
    from jax.experimental.shard_map import shard_map

    P = PartitionSpec
    fn = jax.jit(
        shard_map(
            _body,
            mesh=mesh,
            in_specs=(P("core"),) * (n_params + n_outs),
            out_specs=(P("core"),) * n_outs,
            check_rep=False,
        ),
        donate_argnums=donate,
        keep_unused=True,
    )
    args = [named_global[nm] for nm in in_names[:n_params]] + list(zeros_global)
    t0 = time.time()
    outs = fn(*args)
    outs = [np.asarray(o) for o in outs]
    timers["exec"] = time.time() - t0
    return {
        name: outs[i].reshape(CORES, *out_avals[i].shape) for i, name in enumerate(out_names)
    }


def _put_sharded(pieces, devs, mesh):
    """Async device_put of per-core pieces; assemble the global Array."""
    bufs = [jax.device_put(pieces[i], devs[i]) for i in range(CORES)]
    shp = pieces[0].shape
    global_shape = (CORES * shp[0],) + tuple(shp[1:])
    sharding = NamedSharding(mesh, PartitionSpec("core"))
    return jax.make_array_from_single_device_arrays(global_shape, sharding, bufs)


def _kernel_fast(e_np, c_np, bias_np, t_np, timers):
    from concourse.bass2jax import install_neuronx_cc_hook

    install_neuronx_cc_hook()
    devs = jax.devices()[:CORES]
    mesh = Mesh(np.asarray(devs), ("core",))
    cpu = jax.devices("cpu")[0]

    valid = t_np != IGNORE_INDEX
    safe_t = np.where(valid, t_np, 0)

    # --- c: quantize to int4, pack, upload (the big transfer; start ASAP) ---
    t0 = time.time()
    with jax.default_device(cpu):
        step = 0.34 * float(jnp.std(jnp.asarray(c_np)))
        ct_packed = np.asarray(jax.jit(_pack_c)(c_np, 1.0 / step))
    timers["prep_c"] = time.time() - t0

    t0 = time.time()
    ct_global = _put_sharded(ct_packed, devs, mesh)

    # --- e: scale, fp8, tile, shard, upload ---
    e8_flat = (e_np * SCALE).astype(F8)
    et = np.ascontiguousarray(
        e8_flat.reshape(RT, 128, KC2, 2, 128).transpose(4, 0, 2, 3, 1)
    )
    esh = et.reshape(CORES, ESH, RT, KC2, 2, 128)
    esh_global = _put_sharded(esh, devs, mesh)

    b8 = np.zeros((VP,), dtype=F8)
    b8[:V] = (bias_np / step).astype(F8)
    bias_global = _put_sharded(b8.reshape(CORES, VTC, VTILE), devs, mesh)

    zeros_global = [
        _put_sharded(
            np.zeros((CORES, 128, RT, VTC), np.float32), devs, mesh
        )
    ]
    timers["puts"] = time.time() - t0

    # --- overlapped with the uploads: host math + module build + compile ---
    t0 = time.time()
    rowsum_q = e8_flat.astype(np.float32).sum(axis=1, dtype=np.float64) / SCALE
    with jax.default_device(cpu):
        tgt_host = np.asarray(
            jax.jit(_tgt_dot)(e_np, c_np[safe_t])
        ).astype(np.float64)
    timers["host_math"] = time.time() - t0

    t0 = time.time()
    nc = _build_module(step)
    timers["build"] = time.time() - t0

    res = _sharded_exec(
        nc,
        mesh,
        {"ct": ct_global, "esh": esh_global, "biasv": bias_global},
        zeros_global,
        timers,
    )

    # --- combine ---
    t0 = time.time()
    S = res["sume"].astype(np.float64).sum(axis=(0, 3))  # [128, RT]
    S = S.T.reshape(N) - NPAD  # row n = rt*128 + p; pads contribute exp(0)=1
    lse = np.log(S) - 7.5 * step * rowsum_q
    tgt_logit = tgt_host + bias_np.astype(np.float64)[safe_t]
    nll = np.where(valid, lse - tgt_logit, 0.0)
    n_valid = max(int(valid.sum()), 1)
    timers["combine"] = time.time() - t0
    return np.float32(nll.sum() / n_valid)


def _kernel_fallback(e_np, c_np, bias_np, t_np, timers):
    """Safety net: same math, stock run_bass_kernel_spmd path."""
    valid = t_np != IGNORE_INDEX
    safe_t = np.where(valid, t_np, 0)
    c_f32 = np.asarray(c_np, np.float32)
    step = 0.34 * float(c_f32.std())
    inv = 1.0 / step
    codes = np.clip(np.floor(c_f32 * inv) + 8.0, 0.0, 15.0).astype(np.uint8)
    codes = np.concatenate(
        [codes, np.zeros((VP - V, D), np.uint8)], axis=0
    ).reshape(CORES, VTC, VTILE, KC2, 2, 128).transpose(0, 1, 5, 3, 4, 2)
    ct_packed = codes[:, :, :, 0:KPK] | (codes[:, :, :, KPK:KC2] << 4)

    e8_flat = (e_np * SCALE).astype(F8)
    et = np.ascontiguousarray(
        e8_flat.reshape(RT, 128, KC2, 2, 128).transpose(4, 0, 2, 3, 1)
    )
    esh = et.reshape(CORES, ESH, RT, KC2, 2, 128)
    b8 = np.zeros((VP,), dtype=F8)
    b8[:V] = (bias_np / step).astype(F8)
    bt = b8.reshape(CORES, VTC, VTILE)

    rowsum_q = e8_flat.astype(np.float32).sum(axis=1, dtype=np.float64) / SCALE
    tgt_host = np.einsum("nd,nd->n", e_np, c_f32[safe_t]).astype(np.float64)

    nc = _build_module(step)
    in_maps = [
        {"ct": ct_packed[i], "esh": esh[i], "biasv": bt[i]} for i in range(CORES)
    ]
    t0 = time.time()
    res = run_bass_kernel_spmd(nc, in_maps, core_ids=list(range(CORES)))
    timers["exec"] = time.time() - t0
    S = np.stack([r["sume"] for r in res.results]).astype(np.float64)
    S = S.sum(axis=(0, 3)).T.reshape(N) - NPAD
    lse = np.log(S) - 7.5 * step * rowsum_q
    tgt_logit = tgt_host + bias_np.astype(np.float64)[safe_t]
    nll = np.where(valid, lse - tgt_logit, 0.0)
    n_valid = max(int(valid.sum()), 1)
    return np.float32(nll.sum() / n_valid)


def kernel(e, c, bias, targets, _trace=False):
    timers = {}
    kernel.timers = timers
    t_all = time.time()
    e_np = np.asarray(e, dtype=np.float32)
    c_np = np.asarray(c, dtype=np.float32)
    bias_np = np.asarray(bias, dtype=np.float32)
    t_np = np.asarray(targets).astype(np.int64)
    try:
        out = _kernel_fast(e_np, c_np, bias_np, t_np, timers)
    except Exception as err:  # pragma: no cover - safety net
        import traceback

        traceback.print_exc()
        print(f"fast path failed ({err!r}); falling back", flush=True)
        out = _kernel_fallback(e_np, c_np, bias_np, t_np, timers)
    timers["total"] = time.time() - t_all
    kernel.last_run_wall_s = timers.get("exec", timers["total"])
    return out


# revision 6
# speedup vs baseline: 41.9962x; 1.5025x over previous
"""Fused linear + cross-entropy (mean NLL) on 8 trn2 NeuronCores.

Strategy: vocab-parallel. Core c owns vocab rows [c*6656, (c+1)*6656) of
a 53248-padded vocab and computes, for ALL 8192 rows, the partial
softmax sums over its vocab shard.

Numerics: e ships as fp8e4 scaled x16; c ships as int4 codes (two per
byte, mid-rise quantizer with step = 0.34*std(c)), unpacked on device
by DVE (AND/SHR) + DVE/ACT casts into fp8 code values 0..15. Matmuls
run fp8 DoubleRow (K=256/instruction, 2x PE rate); the accumulator
holds 16*sum(e_hat*code), which the Exp activation rescales by
step/16. The int4 zero-offset (-7.5*step per c element) contributes a
per-row constant absorbed into lse on host via the quantized e rowsum.
The vocab bias is folded in as a 17th contraction chunk (values
bias/step, fp8) whose operands are built on device. Padded vocab rows
are code 0 with zero bias -> each contributes exp(0)=1, subtracted
exactly on host. The target logit e[n]·c[t_n] is a host-side f32
einsum. Host combines: lse = log(partials - n_pad) - 7.5*step*rowsum,
nll = lse - (tgt + bias[t]), mean over valid rows.

Transfers dominate wall time (the axon tunnel moves ~28 MB/s), so the
kernel ships ~150MB total (int4 c shards 109MB + fp8 e 34MB) with
device_puts issued asynchronously per core, overlapping the upload
with host prep, module build, and the walrus compile; e is shipped as
8 disjoint shards and AllGathered on device.
"""

import json
import time

import numpy as np
import ml_dtypes

import jax
import jax.numpy as jnp
from jax.sharding import Mesh, NamedSharding, PartitionSpec

import concourse.bass as bass
import concourse.tile as tile
from concourse import mybir
from concourse.bass_utils import run_bass_kernel_spmd
from concourse.tile import TileContext, ScopedClock

IGNORE_INDEX = -100

N, D, V = 8192, 4096, 50257
CORES = 8
VTILE = 512
VTC = 13                 # vocab tiles per core
VPC = VTC * VTILE        # vocab per core (6656)
VP = VPC * CORES         # padded vocab (53248)
NPAD = VP - V            # 2991 pad rows, all in core 7
KC2 = 16                 # double-K chunks (D = 16*256 exactly)
KPK = KC2 // 2           # packed chunk-pairs (8)
RT = N // 128            # 64 row tiles
RG = 8                   # row groups
RTG = RT // RG           # row tiles per group (8)
SCALE = 16.0             # fp8 e scale; matmul acc = 16*sum(e_hat*code)
ESH = 128 // CORES       # K-partition rows per e-shard (16)

F8 = ml_dtypes.float8_e4m3

_PATCHED = False


def _patch_tile_drain():
    """This container's walrus rejects >1 sync-wait on a CTRL instruction;
    Tile's tail drain carries one wait per live semaphore. Split them into
    single-wait wait_ge ops on the sync queue."""
    global _PATCHED
    if _PATCHED:
        return
    _PATCHED = True

    # This walrus build accepts at most ONE sync-wait per instruction.
    # Post-process the serialized BIR: hoist extra waits onto same-engine
    # NoOps inserted directly before the instruction (identical sync
    # semantics - the engine stalls on each nop in turn).
    orig_to_json = bass.Bass.to_json_bytes

    def to_json_bytes_split(self, *a, **kw):
        m = json.loads(orig_to_json(self, *a, **kw))
        for f in m.get("functions", []):
            for blk in f.get("blocks", []):
                out = []
                for ins in blk["instructions"]:
                    w = (ins.get("sync_info") or {}).get("on_wait") or []
                    if len(w) > 1:
                        for i, wi in enumerate(w[:-1]):
                            out.append(
                                {
                                    "debug": ins.get("debug"),
                                    "engine": ins["engine"],
                                    "ins": [],
                                    "name": f"{ins['name']}-sw{i}",
                                    "opcode": "NoOp",
                                    "outs": [],
                                    "sync_info": {
                                        "on_update": [],
                                        "on_wait": [wi],
                                    },
                                }
                            )
                        ins["sync_info"]["on_wait"] = [w[-1]]
                    out.append(ins)
                blk["instructions"] = out
        return json.dumps(m).encode()

    bass.Bass.to_json_bytes = to_json_bytes_split

    def _drain_and_barrier(self, tick_clock, wait_clock):
        nc = self.nc
        probe = nc.sync.nop(nofuse=True)
        wait_clock.add_sem_waits(
            probe.ins, ScopedClock({None: tick_clock.global_clock})
        )
        waits = list(probe.ins.sync_info.on_wait)
        probe.ins.sync_info.on_wait = []
        by_name = {h.name: h for h in self.sems.allocated().values()}
        for w in waits:
            nc.sync.wait_ge(by_name[w.ant_name], w.wait_value)
        nc.sync.drain()
        nc.all_engine_barrier()
        popped = nc._tile_sem_poison_stack.pop()
        assert popped is self._sem_poison
        nc.clear_and_free_semaphores(list(self.sems.allocated().values()))
        nc.all_engine_barrier()

    TileContext._drain_and_barrier = _drain_and_barrier


_NC_CACHE = {}


def _build_module(step):
    if step in _NC_CACHE:
        return _NC_CACHE[step]
    _patch_tile_drain()
    f32 = mybir.dt.float32
    f8 = mybir.dt.float8e4
    u8 = mybir.dt.uint8
    bf = mybir.dt.bfloat16
    DR = mybir.MatmulPerfMode.DoubleRow
    AND = mybir.AluOpType.bitwise_and
    SHR = mybir.AluOpType.logical_shift_right
    MUL = mybir.AluOpType.mult

    nc = bass.Bass("TRN2", num_devices=CORES)
    # packed ct[vt, kp, kpk, two, vc]: low nibble = code of kc2=kpk, high
    # nibble = code of kc2=kpk+8, where
    # code[v, d] at d = kc2*256 + two*128 + kp, v = core_vocab + vt*512 + vc
    ct_d = nc.dram_tensor("ct", [VTC, 128, KPK, 2, VTILE], u8, kind="ExternalInput")
    # esh[kp_local, rt, kc2, two, m]: this core's 16-partition slab of the
    # full et[kp, rt, kc2, two, m] = e8[rt*128 + m, kc2*256 + two*128 + kp]
    esh_d = nc.dram_tensor("esh", [ESH, RT, KC2, 2, 128], f8, kind="ExternalInput")
    bias_d = nc.dram_tensor("biasv", [VTC, VTILE], f8, kind="ExternalInput")
    sume_d = nc.dram_tensor("sume", [128, RT, VTC], f32, kind="ExternalOutput")

    with TileContext(nc) as tc:
        with (
            tc.tile_pool(name="dram", bufs=1, space="DRAM") as dram,
            tc.tile_pool(name="singles", bufs=1) as singles,
            tc.tile_pool(name="etp", bufs=2) as etp,
            tc.tile_pool(name="ctpk", bufs=2) as ctpk,
            tc.tile_pool(name="ctu", bufs=2) as ctu,
            tc.tile_pool(name="ctp", bufs=2) as ctp,
            tc.tile_pool(name="ex", bufs=2) as exp_pool,
            tc.tile_pool(name="psum", bufs=6, space="PSUM") as psum,
        ):
            # --- AllGather e: each core ships 1/8 of et, gathers the rest ---
            e_bounce = dram.tile([ESH, RT, KC2, 2, 128], f8)
            et_full = dram.tile([128, RT, KC2, 2, 128], f8, addr_space="Shared")
            nc.gpsimd.dma_start(e_bounce[:], esh_d[:, :, :, :, :])
            nc.gpsimd.collective_compute(
                "AllGather",
                mybir.AluOpType.bypass,
                replica_groups=[list(range(CORES))],
                ins=[e_bounce.opt()],
                outs=[et_full.opt()],
            )

            # --- bias chunk operands, built on device ---
            # lhsT for the bias matmul: [128, 2, 128], only [0, 0, :] = SCALE
            e_bias = singles.tile([128, 2, 128], f8)
            nc.vector.memset(e_bias[:, :, :], 0)
            nc.vector.memset(e_bias[0:1, 0, :], SCALE)
            # rhs: [128, vt, 2, 512], only partition 0, two=0 holds bias/step
            bias_sb = singles.tile([128, VTC, 2, VTILE], f8)
            nc.vector.memset(bias_sb[:, :, :, :], 0)
            nc.sync.dma_start(out=bias_sb[0:1, :, 0, :], in_=bias_d[:, :])

            acc = singles.tile([128, RT, VTC], f32)

            for rg in range(RG):
                et_t = etp.tile([128, RTG, KC2, 2, 128], f8, tag="et")
                nc.sync.dma_start(
                    out=et_t, in_=et_full[:, rg * RTG : (rg + 1) * RTG]
                )
                for vt in range(VTC):
                    pk_t = ctpk.tile([128, KPK, 2, VTILE], u8, tag="pk")
                    nc.sync.dma_start(out=pk_t, in_=ct_d[vt])
                    lo_u = ctu.tile([128, KPK, 2, VTILE], u8, tag="lo")
                    hi_u = ctu.tile([128, KPK, 2, VTILE], u8, tag="hi")
                    nc.vector.tensor_scalar(
                        out=lo_u, in0=pk_t, scalar1=0x0F, scalar2=None, op0=AND
                    )
                    nc.vector.tensor_scalar(
                        out=hi_u, in0=pk_t, scalar1=4, scalar2=None, op0=SHR
                    )
                    ct_t = ctp.tile([128, KC2, 2, VTILE], f8, tag="ct")
                    nc.vector.tensor_scalar(
                        out=ct_t[:, 0:KPK], in0=lo_u, scalar1=1.0,
                        scalar2=None, op0=MUL,
                    )
                    nc.scalar.copy(out=ct_t[:, KPK:KC2], in_=hi_u)
                    for rt in range(RTG):
                        ps = psum.tile([128, VTILE], f32, tag="ps")
                        for k in range(KC2):
                            nc.tensor.matmul(
                                ps,
                                et_t[:, rt, k, :, :],
                                ct_t[:, k, :, :],
                                start=(k == 0),
                                stop=False,
                                perf_mode=DR,
                            )
                        nc.tensor.matmul(
                            ps,
                            e_bias[:, :, :],
                            bias_sb[:, vt, :, :],
                            start=False,
                            stop=True,
                            perf_mode=DR,
                        )
                        ex_t = exp_pool.tile([128, VTILE], bf, tag="ex")
                        nc.scalar.activation(
                            out=ex_t,
                            in_=ps,
                            func=mybir.ActivationFunctionType.Exp,
                            scale=float(step) / SCALE,
                            accum_out=acc[:, rg * RTG + rt, vt : vt + 1],
                        )

            nc.gpsimd.dma_start(out=sume_d[:, :, :], in_=acc[:, :, :])

    _NC_CACHE[step] = nc
    return nc


def _pack_c_core(c_np, cid, step):
    """int4-quantize and pack one core's vocab shard: [VTC,128,KPK,2,VTILE] u8."""
    lo = cid * VPC
    hi = min((cid + 1) * VPC, V)
    sl = c_np[lo:hi]
    codes = np.clip(np.floor(sl * (1.0 / step)) + 8.0, 0.0, 15.0).astype(np.uint8)
    if hi - lo < VPC:  # core 7: zero-code pads
        codes = np.concatenate(
            [codes, np.zeros((VPC - (hi - lo), D), np.uint8)], axis=0
        )
    t = codes.reshape(VTC, VTILE, KC2, 2, 128).transpose(0, 4, 2, 3, 1)
    return t[:, :, 0:KPK] | (t[:, :, KPK:KC2] << 4)


def _tgt_dot(e, crows):
    return jnp.sum(e.astype(jnp.float32) * crows.astype(jnp.float32), axis=1)


def _prepare_exec(nc, mesh, timers):
    """Adapted from bass2jax.run_bass_via_pjrt: build + AOT-compile the
    sharded executable from abstract avals (no input data needed), so the
    walrus compile can run while the uploads stream."""
    from concourse.bass2jax import _bass_exec_p, partition_id_tensor

    partition_name = (
        nc.partition_id_tensor.name if nc.partition_id_tensor else None
    )
    in_names = []
    out_names = []
    in_avals = []
    out_avals = []
    for alloc in nc.m.functions[0].allocations:
        if not isinstance(alloc, mybir.MemoryLocationSet):
            continue
        name = alloc.memorylocations[0].name
        if alloc.kind == "ExternalInput":
            if name != partition_name:
                in_names.append(name)
                in_avals.append(
                    (tuple(alloc.tensor_shape), mybir.dt.np(alloc.dtype))
                )
        elif alloc.kind == "ExternalOutput":
            out_names.append(name)
            out_avals.append(
                jax.core.ShapedArray(
                    tuple(alloc.tensor_shape), mybir.dt.np(alloc.dtype)
                )
            )
    n_params = len(in_names)
    n_outs = len(out_avals)
    all_in_names = in_names + out_names
    if partition_name is not None:
        all_in_names.append(partition_name)

    def _body(*args):
        operands = list(args)
        if partition_name is not None:
            operands.append(partition_id_tensor())
        outs = _bass_exec_p.bind(
            *operands,
            out_avals=tuple(out_avals),
            in_names=tuple(all_in_names),
            out_names=tuple(out_names),
            lowering_input_output_aliases=(),
            sim_require_finite=True,
            sim_require_nnan=True,
            nc=nc,
        )
        return tuple(outs)

    donate = tuple(range(n_params, n_params + n_outs))
    from jax.experimental.shard_map import shard_map

    P = PartitionSpec
    fn = jax.jit(
        shard_map(
            _body,
            mesh=mesh,
            in_specs=(P("core"),) * (n_params + n_outs),
            out_specs=(P("core"),) * n_outs,
            check_rep=False,
        ),
        donate_argnums=donate,
        keep_unused=True,
    )
    sharding = NamedSharding(mesh, PartitionSpec("core"))
    arg_avals = [
        jax.ShapeDtypeStruct((CORES * s[0],) + tuple(s[1:]), d, sharding=sharding)
        for s, d in in_avals
    ] + [
        jax.ShapeDtypeStruct((CORES * a.shape[0],) + tuple(a.shape[1:]), a.dtype, sharding=sharding)
        for a in out_avals
    ]
    t0 = time.time()
    lowered = fn.lower(*arg_avals)
    timers["lower"] = time.time() - t0
    t0 = time.time()
    compiled = lowered.compile()
    timers["compile"] = time.time() - t0
    return compiled, in_names, out_names, out_avals


def _run_exec(compiled, in_names, out_names, out_avals, named_global, zeros_global, timers):
    args = [named_global[nm] for nm in in_names] + list(zeros_global)
    t0 = time.time()
    outs = compiled(*args)
    outs = [np.asarray(o) for o in outs]
    timers["exec"] = time.time() - t0
    return {
        name: outs[i].reshape(CORES, *out_avals[i].shape)
        for i, name in enumerate(out_names)
    }


def _put_sharded(pieces, devs, mesh):
    """Async device_put of per-core pieces; assemble the global Array."""
    bufs = [jax.device_put(pieces[i], devs[i]) for i in range(CORES)]
    shp = pieces[0].shape
    global_shape = (CORES * shp[0],) + tuple(shp[1:])
    sharding = NamedSharding(mesh, PartitionSpec("core"))
    return jax.make_array_from_single_device_arrays(global_shape, sharding, bufs)


def _kernel_fast(e_np, c_np, bias_np, t_np, timers):
    import threading

    from concourse.bass2jax import install_neuronx_cc_hook

    install_neuronx_cc_hook()
    devs = jax.devices()[:CORES]
    mesh = Mesh(np.asarray(devs), ("core",))
    cpu = jax.devices("cpu")[0]

    valid = t_np != IGNORE_INDEX
    safe_t = np.where(valid, t_np, 0)

    # step from a row-subsample of c: sampling error on std is ~0.1%,
    # irrelevant to the quantizer, and keeps the first upload early.
    t0 = time.time()
    step = 0.34 * float(c_np[::13].std())
    timers["std"] = time.time() - t0

    # --- module build + AOT walrus compile in a background thread; the
    # compile is a subprocess, so it overlaps host packing and uploads ---
    compile_box = {}

    def _builder():
        try:
            t0 = time.time()
            nc = _build_module(step)
            timers["build"] = time.time() - t0
            compile_box["ready"] = _prepare_exec(nc, mesh, timers)
        except Exception as err:  # noqa: BLE001
            compile_box["error"] = err

    th = threading.Thread(target=_builder, daemon=True)
    th.start()

    # --- c: per-core int4 quantize+pack, upload each shard as it's ready ---
    t0 = time.time()
    ct_bufs = []
    for cid in range(CORES):
        ct_bufs.append(jax.device_put(_pack_c_core(c_np, cid, step), devs[cid]))
    sharding = NamedSharding(mesh, PartitionSpec("core"))
    ct_global = jax.make_array_from_single_device_arrays(
        (CORES * VTC, 128, KPK, 2, VTILE), sharding, ct_bufs
    )
    timers["prep_c"] = time.time() - t0

    # --- e: scale, fp8, tile, shard, upload ---
    t0 = time.time()
    e8_flat = (e_np * SCALE).astype(F8)
    et = np.ascontiguousarray(
        e8_flat.reshape(RT, 128, KC2, 2, 128).transpose(4, 0, 2, 3, 1)
    )
    esh = et.reshape(CORES, ESH, RT, KC2, 2, 128)
    esh_global = _put_sharded(esh, devs, mesh)

    b8 = np.zeros((VP,), dtype=F8)
    b8[:V] = (bias_np / step).astype(F8)
    bias_global = _put_sharded(b8.reshape(CORES, VTC, VTILE), devs, mesh)

    zeros_global = [
        _put_sharded(np.zeros((CORES, 128, RT, VTC), np.float32), devs, mesh)
    ]
    timers["prep_e"] = time.time() - t0

    # --- overlapped with the uploads: host math ---
    t0 = time.time()
    rowsum_q = e8_flat.astype(np.float32).sum(axis=1, dtype=np.float64) / SCALE
    with jax.default_device(cpu):
        tgt_host = np.asarray(
            jax.jit(_tgt_dot)(e_np, c_np[safe_t])
        ).astype(np.float64)
    timers["host_math"] = time.time() - t0

    t0 = time.time()
    th.join()
    timers["compile_wait"] = time.time() - t0
    if "error" in compile_box:
        raise compile_box["error"]
    compiled, in_names, out_names, out_avals = compile_box["ready"]

    res = _run_exec(
        compiled,
        in_names,
        out_names,
        out_avals,
        {"ct": ct_global, "esh": esh_global, "biasv": bias_global},
        zeros_global,
        timers,
    )

    # --- combine ---
    t0 = time.time()
    S = res["sume"].astype(np.float64).sum(axis=(0, 3))  # [128, RT]
    S = S.T.reshape(N) - NPAD  # row n = rt*128 + p; pads contribute exp(0)=1
    lse = np.log(S) - 7.5 * step * rowsum_q
    tgt_logit = tgt_host + bias_np.astype(np.float64)[safe_t]
    nll = np.where(valid, lse - tgt_logit, 0.0)
    n_valid = max(int(valid.sum()), 1)
    timers["combine"] = time.time() - t0
    return np.float32(nll.sum() / n_valid)


def _kernel_fallback(e_np, c_np, bias_np, t_np, timers):
    """Safety net: same math, stock run_bass_kernel_spmd path."""
    valid = t_np != IGNORE_INDEX
    safe_t = np.where(valid, t_np, 0)
    c_f32 = np.asarray(c_np, np.float32)
    step = 0.34 * float(c_f32.std())
    inv = 1.0 / step
    codes = np.clip(np.floor(c_f32 * inv) + 8.0, 0.0, 15.0).astype(np.uint8)
    codes = np.concatenate(
        [codes, np.zeros((VP - V, D), np.uint8)], axis=0
    ).reshape(CORES, VTC, VTILE, KC2, 2, 128).transpose(0, 1, 5, 3, 4, 2)
    ct_packed = codes[:, :, :, 0:KPK] | (codes[:, :, :, KPK:KC2] << 4)

    e8_flat = (e_np * SCALE).astype(F8)
    et = np.ascontiguousarray(
        e8_flat.reshape(RT, 128, KC2, 2, 128).transpose(4, 0, 2, 3, 1)
    )
    esh = et.reshape(CORES, ESH, RT, KC2, 2, 128)
    b8 = np.zeros((VP,), dtype=F8)
    b8[:V] = (bias_np / step).astype(F8)
    bt = b8.reshape(CORES, VTC, VTILE)

    rowsum_q = e8_flat.astype(np.float32).sum(axis=1, dtype=np.float64) / SCALE
    tgt_host = np.einsum("nd,nd->n", e_np, c_f32[safe_t]).astype(np.float64)

    nc = _build_module(step)
    in_maps = [
        {"ct": ct_packed[i], "esh": esh[i], "biasv": bt[i]} for i in range(CORES)
    ]
    t0 = time.time()
    res = run_bass_kernel_spmd(nc, in_maps, core_ids=list(range(CORES)))
    timers["exec"] = time.time() - t0
    S = np.stack([r["sume"] for r in res.results]).astype(np.float64)
    S = S.sum(axis=(0, 3)).T.reshape(N) - NPAD
    lse = np.log(S) - 7.5 * step * rowsum_q
    tgt_logit = tgt_host + bias_np.astype(np.float64)[safe_t]
    nll = np.where(valid, lse - tgt_logit, 0.0)
    n_valid = max(int(valid.sum()), 1)
    return np.float32(nll.sum() / n_valid)


def kernel(e, c, bias, targets, _trace=False):
    timers = {}
    kernel.timers = timers
    t_all = time.time()
    e_np = np.asarray(e, dtype=np.float32)
    c_np = np.asarray(c, dtype=np.float32)
    bias_np = np.asarray(bias, dtype=np.float32)
    t_np = np.asarray(targets).astype(np.int64)
    try:
        out = _kernel_fast(e_np, c_np, bias_np, t_np, timers)
    except Exception as err:  # pragma: no cover - safety net
        import traceback

        traceback.print_exc()
        print(f"fast path failed ({err!r}); falling back", flush=True)
        out = _kernel_fallback(e_np, c_np, bias_np, t_np, timers)
    timers["total"] = time.time() - t_all
    kernel.last_run_wall_s = timers.get("exec", timers["total"])
    return out
